# revision 10
# baseline (speedup 1.0000x reference)
"""Single-head attention (B=4, N=2048, D=1024), scores scaled by 10.

Sharding: 8 cores = (batch, query-half). Core 2b+h owns queries
[1024h:1024(h+1)] of batch b. K/V projections are computed for the OWN
half only and exchanged with the pair core (2b ^ 1) via an on-chip
AllGather, halving the projection FLOPs vs recomputing the full
sequence per core. Key order is global [h0|h1] (rank order) on every
core, so the SPMD program is identical across cores.

Numerics: everything runs as single-pass fp16 matmuls (fp32 PSUM
accumulation). The x10 score scale is folded into the K weights on the
host. Scores are k-partitioned (St tiles) so attention@V consumes P
with no transposes. Per-query max uses a running tensor_max chain
interleaved with the score evacuations, folded across partitions with
DMAs + DVE 32x32 transposes, and broadcast back with a rank-1 matmul.
Softmax sums are exported raw (ones-matmul) and the division happens
on the host, so PV never waits on the sum chain. Output is fp16,
upcast and normalized on the host.

Schedule: weights are host-swizzled (2KB/partition DMA lines) and all
prefetched at t=0. Phase order K, V, Q so the K exchange + readback
(needed first by attention) overlaps V+Q; collective readbacks are
split per n-block so the first score matmuls start while the readback
streams; Q projection's second chunk is emitted between the two score
halves of chunk 0. The attention loop is software-pipelined with
512-wide q-chunks (every matmul is 512 rows, hiding LDWEIGHTS).
"""

import numpy as np

B, SEQ, D = 4, 2048, 1024
NQ = 1024          # queries per core (= keys computed per core)
QCH = 512          # attention q-chunk
NCH = NQ // QCH
NCORES = 8
DT = D // 128      # 8 d-tiles
ET = D // 128      # 8 e-tiles
KT = SEQ // 128    # 16 k-tiles
HKT = KT // 2      # 8 own-half k-tiles

_BUILT = {}


def _build():
    if "nc" in _BUILT:
        return _BUILT["nc"]
    from contextlib import ExitStack

    import concourse.bass as bass  # noqa: F401
    import concourse.mybir as mybir
    import concourse.tile as tile
    from concourse import bacc

    dt = mybir.dt
    F32, F16 = dt.float32, dt.float16
    AL = mybir.AluOpType
    EXP = mybir.ActivationFunctionType.Exp
    GROUPS = [[2 * i, 2 * i + 1] for i in range(NCORES // 2)]

    nc = bacc.Bacc("TRN2", target_bir_lowering=False, debug=False)

    xt_d = nc.dram_tensor("xt", [D, NQ], F16, kind="ExternalInput")
    # weights are host-swizzled: row (blk*128 + p) holds the full 2KB/8KB
    # contraction line for partition p of output-block blk
    wq_d = nc.dram_tensor("wq", [ET * 128, DT * 128], F16, kind="ExternalInput")
    wk_d = nc.dram_tensor("wk", [ET * 128, DT * 128], F16, kind="ExternalInput")
    wv_d = nc.dram_tensor("wv", [2 * 128, DT * 512], F16, kind="ExternalInput")
    ot_d = nc.dram_tensor("ot", [D, NQ], F16, kind="ExternalOutput")
    sm_d = nc.dram_tensor("sm", [1, NQ], F32, kind="ExternalOutput")

    xt_r = xt_d.ap().rearrange("(t p) n -> p t n", p=128)
    wq_r = wq_d.ap().rearrange("(q p) (t e) -> q p t e", p=128, e=128)
    wk_r = wk_d.ap().rearrange("(q p) (t e) -> q p t e", p=128, e=128)
    wv_r = wv_d.ap().rearrange("(q p) (t e) -> q p t e", p=128, e=512)
    ot_r = ot_d.ap().rearrange("(t p) q -> p t q", p=128)

    with tile.TileContext(nc) as tc, ExitStack() as ctx:
        qk_pool = ctx.enter_context(tc.tile_pool(name="qk", bufs=1))
        qt = qk_pool.tile([128, ET, NQ], F16, tag="qt")
        kt = qk_pool.tile([128, ET, SEQ], F16, tag="kt")
        v_pool = ctx.enter_context(tc.tile_pool(name="vp", bufs=1))
        vf = v_pool.tile([128, KT, D], F16, tag="vf")

        const_pool = ctx.enter_context(tc.tile_pool(name="const", bufs=1))
        ones16 = const_pool.tile([128, 1], F16, tag="ones16")
        one32 = const_pool.tile([1, 128], F32, tag="one32")
        nc.vector.memset(ones16[:], 1.0)
        nc.vector.memset(one32[:], 1.0)

        dram = ctx.enter_context(tc.tile_pool(name="dram", bufs=1, space="DRAM"))
        # AllGather outputs have rank blocks [r0, r1] = global key order
        k_in = dram.tile([D, NQ], F16, tag="k_in")
        k_out = dram.tile([2 * D, NQ], F16, tag="k_out")
        v_in = dram.tile([NQ, D], F16, tag="v_in")
        v_out = dram.tile([SEQ, D], F16, tag="v_out")
        warm_in = dram.tile([16, 16], F16, tag="warm_in")
        warm_out = dram.tile([32, 16], F16, tag="warm_out")

        # tiny warmup collective at t=0: pays the ncfw channel-setup latency
        # before the real exchanges need it
        warm_sb = const_pool.tile([16, 16], F16, tag="warm_sb")
        nc.vector.memset(warm_sb[:], 0.0)
        nc.sync.dma_start(warm_in[:], warm_sb[:])
        nc.gpsimd.collective_compute(
            "AllGather",
            AL.bypass,
            replica_groups=GROUPS,
            ins=[warm_in[:]],
            outs=[warm_out[:]],
        )

        # attention-phase pools (declared up front: scores for chunk 0 are
        # emitted while the projection pools are still alive)
        stpool = ctx.enter_context(tc.tile_pool(name="stp", bufs=2))
        treepool = ctx.enter_context(tc.tile_pool(name="tree", bufs=2))
        psS = ctx.enter_context(tc.tile_pool(name="psS", bufs=3, space="PSUM"))
        psO = ctx.enter_context(tc.tile_pool(name="psO", bufs=2, space="PSUM"))
        psX = ctx.enter_context(tc.tile_pool(name="psX", bufs=2, space="PSUM"))
        psR = ctx.enter_context(tc.tile_pool(name="psR", bufs=1, space="PSUM"))

        def scores_part(c, st, rm, kt_lo, kt_hi):
            # scores matmuls with PSUM evacuation and an interleaved
            # running-max chain (k lives on partitions)
            q0 = QCH * c
            for kti in range(kt_lo, kt_hi):
                k0 = 128 * kti
                ps = psS.tile([128, QCH], F32, tag="psS")
                for et in range(ET):
                    nc.tensor.matmul(
                        ps[:],
                        kt[:, et, k0 : k0 + 128],
                        qt[:, et, q0 : q0 + QCH],
                        start=(et == 0),
                        stop=(et == ET - 1),
                    )
                nc.vector.tensor_copy(st[:, kti, :], ps[:])
                if kti == 1:
                    nc.vector.tensor_max(rm[:], st[:, 0, :], st[:, 1, :])
                elif kti > 1:
                    nc.vector.tensor_max(rm[:], rm[:], st[:, kti, :])

        def tree_finish(rm):
            # fold 128 partitions -> 32 (DVE ops need equal start
            # partitions, so move the 32-partition groups with DMAs)
            fold4 = tree1.tile([32, 4, QCH], F32, tag="fold4")
            for a in range(4):
                nc.sync.dma_start(
                    fold4[:, a, :], rm[32 * a : 32 * (a + 1), :]
                )
            nc.vector.tensor_max(fold4[:, 0, :], fold4[:, 0, :], fold4[:, 1, :])
            nc.vector.tensor_max(fold4[:, 2, :], fold4[:, 2, :], fold4[:, 3, :])
            nc.vector.tensor_max(fold4[:, 0, :], fold4[:, 0, :], fold4[:, 2, :])
            t32t = tree1.tile([32, QCH], F32, tag="t32t")
            nc.vector.transpose(t32t[:], fold4[:, 0, :])
            # mx32[r, j] = max over partitions for query q0 + 32j + r
            mx32 = tree1.tile([32, 32], F32, tag="mx32")
            nc.vector.memset(mx32[:], 0.0)
            nc.vector.reduce_max(
                mx32[:, 0 : QCH // 32],
                t32t[:].rearrange("p (j c) -> p j c", c=32),
                axis=mybir.AxisListType.X,
            )
            # transpose once more so q becomes (j-part, r-free) contiguous
            mx32t = tree1.tile([32, 32], F32, tag="mx32t")
            nc.vector.transpose(mx32t[:], mx32[:])
            m1row = tree1.tile([1, QCH], F32, tag="m1row")
            nc.sync.dma_start(m1row[:], mx32t[0 : QCH // 32, :])
            return m1row

        def maxb_mm(m1row):
            maxb_ps = psX.tile([128, QCH], F32, tag="bcast")
            nc.tensor.matmul(
                maxb_ps[:], one32[:], m1row[:], start=True, stop=True
            )
            maxb = auxpool.tile([128, QCH], F32, tag="maxb")
            nc.vector.tensor_copy(maxb[:], maxb_ps[:])
            return maxb

        def exp_stage(c, st, maxb):
            # scores are pre-scaled by 10; exp(s - max) -> fp16 P
            p_t = ppool.tile([128, KT, QCH], F16, tag="p")
            for kti in range(KT):
                nc.vector.scalar_tensor_tensor(
                    st[:, kti, :],
                    st[:, kti, :],
                    1.0,
                    maxb[:],
                    op0=AL.mult,
                    op1=AL.subtract,
                )
                nc.scalar.activation(p_t[:, kti, :], st[:, kti, :], EXP)
            return p_t

        def sum_stage(c, p_t):
            # raw sums over keys via ones-matmul; normalization is on host
            q0 = QCH * c
            sum_ps = psR.tile([1, QCH], F32, tag="sum")
            for kti in range(KT):
                nc.tensor.matmul(
                    sum_ps[:],
                    ones16[:],
                    p_t[:, kti, :],
                    start=(kti == 0),
                    stop=(kti == KT - 1),
                )
            srow = tree1.tile([1, QCH], F32, tag="srow")
            nc.vector.tensor_copy(srow[:], sum_ps[:])
            nc.sync.dma_start(sm_d.ap()[:, q0 : q0 + QCH], srow[:])

        def pv(c, p_t):
            # O^T[d, q] = V^T P (unnormalized; host divides by the sums)
            q0 = QCH * c
            for dti in range(DT):
                d0 = 128 * dti
                ops = psO.tile([128, QCH], F32, tag="psO")
                for kti in range(KT):
                    nc.tensor.matmul(
                        ops[:],
                        vf[:, kti, d0 : d0 + 128],
                        p_t[:, kti, :],
                        start=(kti == 0),
                        stop=(kti == KT - 1),
                    )
                osb = outpool.tile([128, QCH], F16, tag="osb")
                nc.vector.tensor_copy(osb[:], ops[:])
                nc.sync.dma_start(ot_r[:, dti, q0 : q0 + QCH], osb[:])

        with (
            tc.tile_pool(name="xspan", bufs=1) as xspan,
            tc.tile_pool(name="wall", bufs=1) as wall,
            tc.tile_pool(name="wks", bufs=2) as wkpool,
            tc.tile_pool(name="kev", bufs=3) as kevpool,
        ):
            # prefetch everything: first K weight + x (needed first), then
            # the remaining weights in consumption order
            xt_t = xspan.tile([128, DT, NQ], F16, tag="xt")
            wv_t = wall.tile([128, 2, DT, 512], F16, tag="wvt")
            wq_t = wall.tile([128, ET, DT, 128], F16, tag="wqt")
            wk0 = wkpool.tile([128, DT, 128], F16, tag="wk", name="wk0")
            wk1 = wkpool.tile([128, DT, 128], F16, tag="wk", name="wk1")
            nc.sync.dma_start(wk0[:], wk_r[0, :, :, :])
            nc.sync.dma_start(wk1[:], wk_r[1, :, :, :])
            for dti in range(DT):
                nc.sync.dma_start(xt_t[:, dti, :], xt_r[:, dti, :])
            for ec in range(2):
                nc.sync.dma_start(wv_t[:, ec, :, :], wv_r[ec, :, :, :])

            # ------------- Phase K: own-half K^T projection ----------------
            # (wk carries the x10 score scale, folded in on the host)
            wk_cur, wk_nxt = wk0, wk1
            for et in range(ET):
                e0 = 128 * et
                if 0 < et < ET - 1:
                    wk_nxt = wkpool.tile(
                        [128, DT, 128], F16, tag="wk", name=f"wk{et+1}"
                    )
                    nc.sync.dma_start(wk_nxt[:], wk_r[et + 1, :, :, :])
                for chn in range(NQ // 512):
                    n0 = 512 * chn
                    ps = psS.tile([128, 512], F32, tag="psS")
                    for dti in range(DT):
                        nc.tensor.matmul(
                            ps[:],
                            wk_cur[:, dti, :],
                            xt_t[:, dti, n0 : n0 + 512],
                            start=(dti == 0),
                            stop=(dti == DT - 1),
                        )
                    kev = kevpool.tile([128, 512], F16, tag="kev")
                    nc.vector.tensor_copy(kev[:], ps[:])
                    nc.sync.dma_start(k_in[e0 : e0 + 128, n0 : n0 + 512], kev[:])
                wk_cur = wk_nxt

            for et in range(ET):
                nc.sync.dma_start(wq_t[:, et, :, :], wq_r[et, :, :, :])

            nc.gpsimd.collective_compute(
                "AllGather",
                AL.bypass,
                replica_groups=GROUPS,
                ins=[k_in[:]],
                outs=[k_out[:]],
            )
            # fine-grained readback (per key-block) so the first score
            # matmuls start while the readback is still streaming
            k_out_r = k_out[:].rearrange("(b t p) n -> b p t n", p=128, t=ET)
            for h in range(2):
                for nb in range(2):
                    n0 = 512 * nb
                    nc.gpsimd.dma_start(
                        kt[:, :, NQ * h + n0 : NQ * h + n0 + 512],
                        k_out_r[h, :, :, n0 : n0 + 512],
                    )

            # ------------- Phase V: own-half V projection ------------------
            for ec in range(2):
                e0 = 512 * ec
                for kti in range(HKT):
                    k0 = 128 * kti
                    ps = psS.tile([128, 512], F32, tag="psS")
                    for dti in range(DT):
                        nc.tensor.matmul(
                            ps[:],
                            xt_t[:, dti, k0 : k0 + 128],
                            wv_t[:, ec, dti, :],
                            start=(dti == 0),
                            stop=(dti == DT - 1),
                        )
                    vev = kevpool.tile([128, 512], F16, tag="vev")
                    nc.vector.tensor_copy(vev[:], ps[:])
                    nc.sync.dma_start(
                        v_in[k0 : k0 + 128, e0 : e0 + 512], vev[:]
                    )

            nc.gpsimd.collective_compute(
                "AllGather",
                AL.bypass,
                replica_groups=GROUPS,
                ins=[v_in[:]],
                outs=[v_out[:]],
            )
            v_out_r = v_out[:].rearrange("(b t p) e -> b p t e", p=128, t=HKT)
            for h in range(2):
                for kti in range(HKT):
                    nc.gpsimd.dma_start(
                        vf[:, HKT * h + kti, :], v_out_r[h, :, kti, :]
                    )

            # ------------- Phase Q: own-half Q^T projection ----------------
            # chunk-major so chunk 0's queries are ready at half-phase, with
            # chunk 0's first score matmuls emitted in between
            def qproj(chn):
                n0 = 512 * chn
                for et in range(ET):
                    ps = psS.tile([128, 512], F32, tag="psS")
                    for dti in range(DT):
                        nc.tensor.matmul(
                            ps[:],
                            wq_t[:, et, dti, :],
                            xt_t[:, dti, n0 : n0 + 512],
                            start=(dti == 0),
                            stop=(dti == DT - 1),
                        )
                    nc.vector.tensor_copy(qt[:, et, n0 : n0 + 512], ps[:])

            sts = [None] * NCH
            rms = [None] * NCH
            sts[0] = stpool.tile([128, KT, QCH], F32, tag="st", name="st0")
            rms[0] = treepool.tile([128, QCH], F32, tag="rm", name="rm0")
            qproj(0)
            scores_part(0, sts[0], rms[0], 0, KT // 2)
            qproj(1)
            scores_part(0, sts[0], rms[0], KT // 2, KT)

        # ---------------- Phase B: attention, q-chunked -------------------
        # (these pools reuse the space freed by the projection pools)
        ppool = ctx.enter_context(tc.tile_pool(name="pp", bufs=2))
        tree1 = ctx.enter_context(tc.tile_pool(name="tree1", bufs=1))
        auxpool = ctx.enter_context(tc.tile_pool(name="aux", bufs=2))
        outpool = ctx.enter_context(tc.tile_pool(name="osb", bufs=2))
        m1row = tree_finish(rms[0])
        for c in range(NCH):
            nxt = c + 1
            if nxt < NCH:
                sts[nxt] = stpool.tile(
                    [128, KT, QCH], F32, tag="st", name=f"st{nxt}"
                )
                rms[nxt] = treepool.tile(
                    [128, QCH], F32, tag="rm", name=f"rm{nxt}"
                )
                scores_part(nxt, sts[nxt], rms[nxt], 0, KT // 2)
            maxb = maxb_mm(m1row)
            p_t = exp_stage(c, sts[c], maxb)
            if nxt < NCH:
                scores_part(nxt, sts[nxt], rms[nxt], KT // 2, KT)
                m1row = tree_finish(rms[nxt])
            pv(c, p_t)
            sum_stage(c, p_t)

    nc.compile()
    _BUILT["nc"] = nc
    return nc


def _prep_inputs(x, q_w, k_w, v_w):
    def swz(wt, ew):
        # [D, D] w.T -> rows of (e-block, partition) holding contiguous
        # [DT, ew] contraction lines
        a = wt.reshape(DT, 128, D // ew, ew)
        return np.ascontiguousarray(
            a.transpose(2, 1, 0, 3).reshape(D // ew * 128, DT * ew)
        ).astype(np.float16)

    wq = swz(q_w.T, 128)
    # fold the x10 score scale into K (fp16 relative precision unchanged)
    wk = swz(k_w.T * 10.0, 128)
    wv = swz(v_w.T, 512)

    in_maps = []
    for core in range(NCORES):
        b, h = divmod(core, 2)
        xt = np.ascontiguousarray(
            np.asarray(x[b, NQ * h : NQ * (h + 1)]).T
        ).astype(np.float16)
        in_maps.append({"xt": xt, "wq": wq, "wk": wk, "wv": wv})
    return in_maps


def run(x, q_w, k_w, v_w, trace=False):
    from concourse.bass_utils import run_bass_kernel_spmd

    nc = _build()
    in_maps = _prep_inputs(x, q_w, k_w, v_w)
    res = run_bass_kernel_spmd(nc, in_maps, list(range(NCORES)), trace=trace)
    out = np.empty((B, SEQ, D), np.float32)
    for core in range(NCORES):
        b, h = divmod(core, 2)
        ot = res.results[core]["ot"].T.astype(np.float32)
        sums = res.results[core]["sm"].reshape(NQ, 1).astype(np.float32)
        out[b, NQ * h : NQ * (h + 1)] = ot / sums
    return out, res


def kernel(x, q_w, k_w, v_w):
    x = np.asarray(x, np.float32)
    q_w = np.asarray(q_w, np.float32)
    k_w = np.asarray(k_w, np.float32)
    v_w = np.asarray(v_w, np.float32)
    out, _ = run(x, q_w, k_w, v_w, trace=False)
    return out


# revision 11
# speedup vs baseline: 1.0931x; 1.0931x over previous
"""Single-head attention (B=4, N=2048, D=1024), scores scaled by 10.

Sharding: 8 cores = (batch, query-half). Core 2b+h owns queries
[1024h:1024(h+1)] of batch b. K/V projections are computed for the OWN
half only and exchanged with the pair core (2b ^ 1) via an on-chip
AllGather, halving the projection FLOPs vs recomputing the full
sequence per core. Key order is global [h0|h1] (rank order) on every
core, so the SPMD program is identical across cores.

Numerics: everything runs as single-pass fp16 matmuls (fp32 PSUM
accumulation). The x10 score scale is folded into the K weights on the
host. Scores are k-partitioned (St tiles) so attention@V consumes P
with no transposes. Per-query max uses a running tensor_max chain
interleaved with the score evacuations, folded across partitions with
DMAs + DVE 32x32 transposes, and broadcast back with a rank-1 matmul.
Softmax sums are exported raw (ones-matmul) and the division happens
on the host, so PV never waits on the sum chain. Output is fp16,
upcast and normalized on the host.

Schedule: weights are host-swizzled (2KB/partition DMA lines) and all
prefetched at t=0. Phase order K, V, Q so the K exchange + readback
(needed first by attention) overlaps V+Q; collective readbacks are
split per n-block so the first score matmuls start while the readback
streams; Q projection's second chunk is emitted between the two score
halves of chunk 0. The attention loop is software-pipelined with
512-wide q-chunks (every matmul is 512 rows, hiding LDWEIGHTS).
"""

import numpy as np

B, SEQ, D = 4, 2048, 1024
NQ = 1024          # queries per core (= keys computed per core)
QCH = 512          # attention q-chunk
NCH = NQ // QCH
NCORES = 8
DT = D // 128      # 8 d-tiles
ET = D // 128      # 8 e-tiles
KT = SEQ // 128    # 16 k-tiles
HKT = KT // 2      # 8 own-half k-tiles

_BUILT = {}


def _build():
    if "nc" in _BUILT:
        return _BUILT["nc"]
    from contextlib import ExitStack

    import concourse.bass as bass  # noqa: F401
    import concourse.mybir as mybir
    import concourse.tile as tile
    from concourse import bacc

    dt = mybir.dt
    F32, F16 = dt.float32, dt.float16
    AL = mybir.AluOpType
    EXP = mybir.ActivationFunctionType.Exp
    GROUPS = [[2 * i, 2 * i + 1] for i in range(NCORES // 2)]

    nc = bacc.Bacc("TRN2", target_bir_lowering=False, debug=False)

    xt_d = nc.dram_tensor("xt", [D, NQ], F16, kind="ExternalInput")
    # weights are host-swizzled: row (blk*128 + p) holds the full 2KB/8KB
    # contraction line for partition p of output-block blk
    wq_d = nc.dram_tensor("wq", [ET * 128, DT * 128], F16, kind="ExternalInput")
    wk_d = nc.dram_tensor("wk", [ET * 128, DT * 128], F16, kind="ExternalInput")
    wv_d = nc.dram_tensor("wv", [2 * 128, DT * 512], F16, kind="ExternalInput")
    ot_d = nc.dram_tensor("ot", [D, NQ], F16, kind="ExternalOutput")
    sm_d = nc.dram_tensor("sm", [1, NQ], F32, kind="ExternalOutput")

    xt_r = xt_d.ap().rearrange("(t p) n -> p t n", p=128)
    wq_r = wq_d.ap().rearrange("(q p) (t e) -> q p t e", p=128, e=128)
    wk_r = wk_d.ap().rearrange("(q p) (t e) -> q p t e", p=128, e=128)
    wv_r = wv_d.ap().rearrange("(q p) (t e) -> q p t e", p=128, e=512)
    ot_r = ot_d.ap().rearrange("(t p) q -> p t q", p=128)

    with tile.TileContext(nc) as tc, ExitStack() as ctx:
        qk_pool = ctx.enter_context(tc.tile_pool(name="qk", bufs=1))
        qt = qk_pool.tile([128, ET, NQ], F16, tag="qt")
        kt = qk_pool.tile([128, ET, SEQ], F16, tag="kt")
        v_pool = ctx.enter_context(tc.tile_pool(name="vp", bufs=1))
        vf = v_pool.tile([128, KT, D], F16, tag="vf")

        const_pool = ctx.enter_context(tc.tile_pool(name="const", bufs=1))
        ones16 = const_pool.tile([128, 1], F16, tag="ones16")
        one32 = const_pool.tile([1, 128], F32, tag="one32")
        nc.vector.memset(ones16[:], 1.0)
        nc.vector.memset(one32[:], 1.0)

        dram = ctx.enter_context(tc.tile_pool(name="dram", bufs=1, space="DRAM"))
        # AllGather outputs have rank blocks [r0, r1] = global key order
        k_in = dram.tile([D, NQ], F16, tag="k_in")
        k_out = dram.tile([2 * D, NQ], F16, tag="k_out")
        v_in = dram.tile([NQ, D], F16, tag="v_in")
        v_out = dram.tile([SEQ, D], F16, tag="v_out")
        warm_in = dram.tile([16, 16], F16, tag="warm_in")
        warm_out = dram.tile([32, 16], F16, tag="warm_out")

        # tiny warmup collective at t=0: pays the ncfw channel-setup latency
        # before the real exchanges need it
        warm_sb = const_pool.tile([16, 16], F16, tag="warm_sb")
        nc.vector.memset(warm_sb[:], 0.0)
        nc.sync.dma_start(warm_in[:], warm_sb[:])
        nc.gpsimd.collective_compute(
            "AllGather",
            AL.bypass,
            replica_groups=GROUPS,
            ins=[warm_in[:]],
            outs=[warm_out[:]],
        )

        # attention-phase pools (declared up front: scores for chunk 0 are
        # emitted while the projection pools are still alive)
        stpool = ctx.enter_context(tc.tile_pool(name="stp", bufs=2))
        treepool = ctx.enter_context(tc.tile_pool(name="tree", bufs=2))
        psS = ctx.enter_context(tc.tile_pool(name="psS", bufs=3, space="PSUM"))
        psO = ctx.enter_context(tc.tile_pool(name="psO", bufs=2, space="PSUM"))
        psX = ctx.enter_context(tc.tile_pool(name="psX", bufs=2, space="PSUM"))
        psR = ctx.enter_context(tc.tile_pool(name="psR", bufs=1, space="PSUM"))

        def scores_part(c, st, rm, kt_lo, kt_hi):
            # scores matmuls with PSUM evacuation and an interleaved
            # running-max chain (k lives on partitions)
            q0 = QCH * c
            for kti in range(kt_lo, kt_hi):
                k0 = 128 * kti
                ps = psS.tile([128, QCH], F32, tag="psS")
                for et in range(ET):
                    nc.tensor.matmul(
                        ps[:],
                        kt[:, et, k0 : k0 + 128],
                        qt[:, et, q0 : q0 + QCH],
                        start=(et == 0),
                        stop=(et == ET - 1),
                    )
                nc.vector.tensor_copy(st[:, kti, :], ps[:])
                if kti == 1:
                    nc.vector.tensor_max(rm[:], st[:, 0, :], st[:, 1, :])
                elif kti > 1:
                    nc.vector.tensor_max(rm[:], rm[:], st[:, kti, :])

        def tree_finish(rm):
            # fold 128 partitions -> 32 (DVE ops need equal start
            # partitions, so move the 32-partition groups with DMAs)
            fold4 = tree1.tile([32, 4, QCH], F32, tag="fold4")
            for a in range(4):
                nc.sync.dma_start(
                    fold4[:, a, :], rm[32 * a : 32 * (a + 1), :]
                )
            nc.vector.tensor_max(fold4[:, 0, :], fold4[:, 0, :], fold4[:, 1, :])
            nc.vector.tensor_max(fold4[:, 2, :], fold4[:, 2, :], fold4[:, 3, :])
            nc.vector.tensor_max(fold4[:, 0, :], fold4[:, 0, :], fold4[:, 2, :])
            t32t = tree1.tile([32, QCH], F32, tag="t32t")
            nc.vector.transpose(t32t[:], fold4[:, 0, :])
            # mx32[r, j] = max over partitions for query q0 + 32j + r
            mx32 = tree1.tile([32, 32], F32, tag="mx32")
            nc.vector.memset(mx32[:], 0.0)
            nc.vector.reduce_max(
                mx32[:, 0 : QCH // 32],
                t32t[:].rearrange("p (j c) -> p j c", c=32),
                axis=mybir.AxisListType.X,
            )
            # transpose once more so q becomes (j-part, r-free) contiguous
            mx32t = tree1.tile([32, 32], F32, tag="mx32t")
            nc.vector.transpose(mx32t[:], mx32[:])
            m1row = tree1.tile([1, QCH], F32, tag="m1row")
            nc.sync.dma_start(m1row[:], mx32t[0 : QCH // 32, :])
            return m1row

        def maxb_mm(m1row):
            maxb_ps = psX.tile([128, QCH], F32, tag="bcast")
            nc.tensor.matmul(
                maxb_ps[:], one32[:], m1row[:], start=True, stop=True
            )
            maxb = auxpool.tile([128, QCH], F32, tag="maxb")
            nc.vector.tensor_copy(maxb[:], maxb_ps[:])
            return maxb

        def exp_stage(c, st, maxb):
            # scores are pre-scaled by 10; exp(s - max) -> fp16 P
            p_t = ppool.tile([128, KT, QCH], F16, tag="p")
            for kti in range(KT):
                nc.vector.scalar_tensor_tensor(
                    st[:, kti, :],
                    st[:, kti, :],
                    1.0,
                    maxb[:],
                    op0=AL.mult,
                    op1=AL.subtract,
                )
                nc.scalar.activation(p_t[:, kti, :], st[:, kti, :], EXP)
            return p_t

        def sum_stage(c, p_t):
            # raw sums over keys via ones-matmul; normalization is on host
            q0 = QCH * c
            sum_ps = psR.tile([1, QCH], F32, tag="sum")
            for kti in range(KT):
                nc.tensor.matmul(
                    sum_ps[:],
                    ones16[:],
                    p_t[:, kti, :],
                    start=(kti == 0),
                    stop=(kti == KT - 1),
                )
            srow = tree1.tile([1, QCH], F32, tag="srow")
            nc.vector.tensor_copy(srow[:], sum_ps[:])
            nc.sync.dma_start(sm_d.ap()[:, q0 : q0 + QCH], srow[:])

        def pv(c, p_t):
            # O^T[d, q] = V^T P (unnormalized; host divides by the sums)
            q0 = QCH * c
            for dti in range(DT):
                d0 = 128 * dti
                ops = psO.tile([128, QCH], F32, tag="psO")
                for kti in range(KT):
                    nc.tensor.matmul(
                        ops[:],
                        vf[:, kti, d0 : d0 + 128],
                        p_t[:, kti, :],
                        start=(kti == 0),
                        stop=(kti == KT - 1),
                    )
                osb = outpool.tile([128, QCH], F16, tag="osb")
                nc.vector.tensor_copy(osb[:], ops[:])
                nc.sync.dma_start(ot_r[:, dti, q0 : q0 + QCH], osb[:])

        with (
            tc.tile_pool(name="xspan", bufs=1) as xspan,
            tc.tile_pool(name="wall", bufs=1) as wall,
            tc.tile_pool(name="wks", bufs=2) as wkpool,
            tc.tile_pool(name="kev", bufs=3) as kevpool,
        ):
            # prefetch everything: first K weight + x (needed first), then
            # the remaining weights in consumption order
            xt_t = xspan.tile([128, DT, NQ], F16, tag="xt")
            wv_t = wall.tile([128, 2, DT, 512], F16, tag="wvt")
            wq_t = wall.tile([128, ET, DT, 128], F16, tag="wqt")
            wk0 = wkpool.tile([128, DT, 128], F16, tag="wk", name="wk0")
            wk1 = wkpool.tile([128, DT, 128], F16, tag="wk", name="wk1")
            nc.sync.dma_start(wk0[:], wk_r[0, :, :, :])
            nc.sync.dma_start(wk1[:], wk_r[1, :, :, :])
            for dti in range(DT):
                nc.sync.dma_start(xt_t[:, dti, :], xt_r[:, dti, :])
            for ec in range(2):
                nc.sync.dma_start(wv_t[:, ec, :, :], wv_r[ec, :, :, :])
            for et in range(ET):
                nc.sync.dma_start(wq_t[:, et, :, :], wq_r[et, :, :, :])

            # ------------- Phase K: own-half K^T projection ----------------
            # (wk carries the x10 score scale, folded in on the host)
            wk_cur, wk_nxt = wk0, wk1
            for et in range(ET):
                e0 = 128 * et
                if 0 < et < ET - 1:
                    wk_nxt = wkpool.tile(
                        [128, DT, 128], F16, tag="wk", name=f"wk{et+1}"
                    )
                    nc.sync.dma_start(wk_nxt[:], wk_r[et + 1, :, :, :])
                for chn in range(NQ // 512):
                    n0 = 512 * chn
                    ps = psS.tile([128, 512], F32, tag="psS")
                    for dti in range(DT):
                        nc.tensor.matmul(
                            ps[:],
                            wk_cur[:, dti, :],
                            xt_t[:, dti, n0 : n0 + 512],
                            start=(dti == 0),
                            stop=(dti == DT - 1),
                        )
                    kev = kevpool.tile([128, 512], F16, tag="kev")
                    nc.vector.tensor_copy(kev[:], ps[:])
                    nc.sync.dma_start(k_in[e0 : e0 + 128, n0 : n0 + 512], kev[:])
                wk_cur = wk_nxt

            nc.gpsimd.collective_compute(
                "AllGather",
                AL.bypass,
                replica_groups=GROUPS,
                ins=[k_in[:]],
                outs=[k_out[:]],
            )
            # fine-grained readback (per key-block) so the first score
            # matmuls start while the readback is still streaming
            k_out_r = k_out[:].rearrange("(b t p) n -> b p t n", p=128, t=ET)
            for h in range(2):
                for nb in range(2):
                    n0 = 512 * nb
                    nc.gpsimd.dma_start(
                        kt[:, :, NQ * h + n0 : NQ * h + n0 + 512],
                        k_out_r[h, :, :, n0 : n0 + 512],
                    )

            # ------------- Phase V: own-half V projection ------------------
            for ec in range(2):
                e0 = 512 * ec
                for kti in range(HKT):
                    k0 = 128 * kti
                    ps = psS.tile([128, 512], F32, tag="psS")
                    for dti in range(DT):
                        nc.tensor.matmul(
                            ps[:],
                            xt_t[:, dti, k0 : k0 + 128],
                            wv_t[:, ec, dti, :],
                            start=(dti == 0),
                            stop=(dti == DT - 1),
                        )
                    vev = kevpool.tile([128, 512], F16, tag="vev")
                    nc.vector.tensor_copy(vev[:], ps[:])
                    nc.sync.dma_start(
                        v_in[k0 : k0 + 128, e0 : e0 + 512], vev[:]
                    )

            nc.gpsimd.collective_compute(
                "AllGather",
                AL.bypass,
                replica_groups=GROUPS,
                ins=[v_in[:]],
                outs=[v_out[:]],
            )
            v_out_r = v_out[:].rearrange("(b t p) e -> b p t e", p=128, t=HKT)
            for h in range(2):
                for kti in range(HKT):
                    nc.gpsimd.dma_start(
                        vf[:, HKT * h + kti, :], v_out_r[h, :, kti, :]
                    )

            # ------------- Phase Q: own-half Q^T projection ----------------
            # chunk-major so chunk 0's queries are ready at half-phase, with
            # chunk 0's first score matmuls emitted in between
            def qproj(chn):
                n0 = 512 * chn
                for et in range(ET):
                    ps = psS.tile([128, 512], F32, tag="psS")
                    for dti in range(DT):
                        nc.tensor.matmul(
                            ps[:],
                            wq_t[:, et, dti, :],
                            xt_t[:, dti, n0 : n0 + 512],
                            start=(dti == 0),
                            stop=(dti == DT - 1),
                        )
                    nc.vector.tensor_copy(qt[:, et, n0 : n0 + 512], ps[:])

            sts = [None] * NCH
            rms = [None] * NCH
            sts[0] = stpool.tile([128, KT, QCH], F32, tag="st", name="st0")
            rms[0] = treepool.tile([128, QCH], F32, tag="rm", name="rm0")
            qproj(0)
            scores_part(0, sts[0], rms[0], 0, KT // 2)
            qproj(1)
            scores_part(0, sts[0], rms[0], KT // 2, KT)

        # ---------------- Phase B: attention, q-chunked -------------------
        # (these pools reuse the space freed by the projection pools)
        ppool = ctx.enter_context(tc.tile_pool(name="pp", bufs=2))
        tree1 = ctx.enter_context(tc.tile_pool(name="tree1", bufs=1))
        auxpool = ctx.enter_context(tc.tile_pool(name="aux", bufs=2))
        outpool = ctx.enter_context(tc.tile_pool(name="osb", bufs=2))
        m1row = tree_finish(rms[0])
        for c in range(NCH):
            nxt = c + 1
            if nxt < NCH:
                sts[nxt] = stpool.tile(
                    [128, KT, QCH], F32, tag="st", name=f"st{nxt}"
                )
                rms[nxt] = treepool.tile(
                    [128, QCH], F32, tag="rm", name=f"rm{nxt}"
                )
                scores_part(nxt, sts[nxt], rms[nxt], 0, KT // 2)
            maxb = maxb_mm(m1row)
            p_t = exp_stage(c, sts[c], maxb)
            if nxt < NCH:
                scores_part(nxt, sts[nxt], rms[nxt], KT // 2, KT)
                m1row = tree_finish(rms[nxt])
            pv(c, p_t)
            sum_stage(c, p_t)

    nc.compile()
    _BUILT["nc"] = nc
    return nc


def _prep_inputs(x, q_w, k_w, v_w):
    def swz(wt, ew):
        # [D, D] w.T -> rows of (e-block, partition) holding contiguous
        # [DT, ew] contraction lines
        a = wt.reshape(DT, 128, D // ew, ew)
        return np.ascontiguousarray(
            a.transpose(2, 1, 0, 3).reshape(D // ew * 128, DT * ew)
        ).astype(np.float16)

    wq = swz(q_w.T, 128)
    # fold the x10 score scale into K (fp16 relative precision unchanged)
    wk = swz(k_w.T * 10.0, 128)
    wv = swz(v_w.T, 512)

    in_maps = []
    for core in range(NCORES):
        b, h = divmod(core, 2)
        xt = np.ascontiguousarray(
            np.asarray(x[b, NQ * h : NQ * (h + 1)]).T
        ).astype(np.float16)
        in_maps.append({"xt": xt, "wq": wq, "wk": wk, "wv": wv})
    return in_maps


def run(x, q_w, k_w, v_w, trace=False):
    from concourse.bass_utils import run_bass_kernel_spmd

    nc = _build()
    in_maps = _prep_inputs(x, q_w, k_w, v_w)
    res = run_bass_kernel_spmd(nc, in_maps, list(range(NCORES)), trace=trace)
    out = np.empty((B, SEQ, D), np.float32)
    for core in range(NCORES):
        b, h = divmod(core, 2)
        ot = res.results[core]["ot"].T.astype(np.float32)
        sums = res.results[core]["sm"].reshape(NQ, 1).astype(np.float32)
        out[b, NQ * h : NQ * (h + 1)] = ot / sums
    return out, res


def kernel(x, q_w, k_w, v_w):
    x = np.asarray(x, np.float32)
    q_w = np.asarray(q_w, np.float32)
    k_w = np.asarray(k_w, np.float32)
    v_w = np.asarray(v_w, np.float32)
    out, _ = run(x, q_w, k_w, v_w, trace=False)
    return out
